# revision 26
# baseline (speedup 1.0000x reference)
"""Trainium2 Bass kernel for nn_BiMamba (linear recurrence, last-step output).

Reference computes
    u = x @ input_matrix                       # [B, T, D]
    h_t = h_{t-1} @ state_matrix + u_t         # scan over T
    out = h_{T-1} @ output_matrix              # [B, 1]

Because only the LAST timestep's output is read, the scan collapses exactly:
    out[b] = sum_t  x[b,t,:] . W[t,:],      W[t,:] = B_in @ A^(T-1-t) @ C

W is a tiny [T, D] matrix computed on the host in float64 (a length-T chain of
D x D matvecs, ~270 MFLOP).  The device kernel is then a pure memory-bound
weighted reduction over x, data-parallel over batch across the 8 NeuronCores:
VectorE forms prod = x * W (W broadcast across batches via a stride-0 access
pattern) and reduces along the free axis; the final 128-way partition sums
(a [128, 8] tensor per core) are done on the host.

Because A = PARAM_SCALE * randn (spectral norm ~0.32), W[t] decays as
0.32^(T-1-t): every W row older than the trailing ~64 steps underflows to an
EXACT 0.0 in float32.  Rows that are exactly zero contribute exactly zero to
the float32 reduction, so the device only needs the trailing slice of x that
covers W's nonzero support.  The kernel checks this numerically on the host
each call and picks the smallest compiled bucket whose trailing window covers
every nonzero float32 row of W; if the parameters ever stopped decaying it
falls back to the full T=2048 window.  The result is identical (up to f32
summation order) to processing all of x.
"""

import os
from contextlib import ExitStack

import numpy as np

B_FULL = 64
T = 2048
D = 256
N_CORES = 8
B_LOC = B_FULL // N_CORES  # 8 batches per core
P = 128                    # SBUF partitions

# trailing-window buckets (timesteps); each has its own compiled NEFF
BUCKETS = (64, 128, 256, 2048)
# batch split across the input DMAs of the small-bucket design: equal small
# chunks so data arrivals pipeline tightly with the DVE multiply+reduce chain
_CHUNKS = (2, 2, 2, 2)

_CACHE = {}
LAST_RESULTS = None  # BassKernelResults of the most recent run (for test.py)


def _compute_w(state_matrix, input_matrix, output_matrix) -> np.ndarray:
    """W[t, :] = input_matrix @ state_matrix^(T-1-t) @ output_matrix, f64."""
    A = np.asarray(state_matrix, dtype=np.float64)
    Bm = np.asarray(input_matrix, dtype=np.float64)
    C = np.asarray(output_matrix, dtype=np.float64).reshape(D)
    V = np.empty((T, D), dtype=np.float64)
    v = C.copy()
    for i in range(T):
        V[T - 1 - i] = v
        v = A @ v
    return V @ Bm.T  # [T, D] f64


def _pick_bucket(w32: np.ndarray) -> int:
    """Smallest bucket whose trailing window covers all nonzero f32 W rows."""
    for keep in BUCKETS:
        if keep >= T or not np.any(w32[: T - keep]):
            return min(keep, T)
    return T


def _build_bass(keep: int):
    import concourse.bacc as bacc
    import concourse.mybir as mybir
    import concourse.tile as tile

    free = keep * D // P          # free-dim elems per partition per batch

    nc = bacc.Bacc("TRN2", target_bir_lowering=False, debug=False,
                   num_devices=N_CORES)
    f32 = mybir.dt.float32

    if keep <= 256:
        # grouped design: one input DMA per batch chunk, W packed in front
        # of chunk 0; compute on chunk g overlaps the DMA of chunk g+1.
        xts = []
        for i, nb in enumerate(_CHUNKS):
            cols = ((1 + nb) if i == 0 else nb) * free
            xts.append(nc.dram_tensor(f"xs{i}", [P, cols], f32,
                                      kind="ExternalInput"))
        out = nc.dram_tensor("out", [P, B_LOC], f32, kind="ExternalOutput")

        with ExitStack() as ctx:
            tc = ctx.enter_context(tile.TileContext(nc))
            pool = ctx.enter_context(tc.tile_pool(name="pool", bufs=1))
            ppool = ctx.enter_context(tc.tile_pool(name="ppool", bufs=2))

            tiles = []
            for i, dram in enumerate(xts):
                t = pool.tile(list(dram.shape), f32, tag=f"t{i}")
                nc.sync.dma_start(t[:], dram.ap())
                tiles.append(t)

            wt = tiles[0][:, :free].rearrange("p (one f) -> p one f", one=1)
            res = pool.tile([P, B_LOC], f32, tag="res")

            groups = [(tiles[0][:, free:], _CHUNKS[0])] + [
                (tiles[i][:], nb) for i, nb in enumerate(_CHUNKS) if i > 0]
            col = 0
            for g, (xg, nb) in enumerate(groups):
                prod = ppool.tile([P, nb, free], f32, tag=f"prod{g}")
                nc.vector.tensor_mul(
                    prod[:], xg.rearrange("p (nb f) -> p nb f", f=free),
                    wt.broadcast_to((P, nb, free)))
                nc.vector.reduce_sum(res[:, col:col + nb], prod[:],
                                     axis=mybir.AxisListType.X)
                col += nb

            nc.sync.dma_start(out[:], res[:])
        nc.compile()
        return nc

    # full-window fallback: per-batch pipeline, DVE multiply + ACT reduce
    chunk = min(free, 2048)
    nch = free // chunk
    xs = nc.dram_tensor("xs", [B_LOC, nch, P, chunk], f32,
                        kind="ExternalInput")
    w = nc.dram_tensor("w", [nch, P, chunk], f32, kind="ExternalInput")
    out = nc.dram_tensor("out", [P, B_LOC * nch], f32, kind="ExternalOutput")

    with ExitStack() as ctx:
        tc = ctx.enter_context(tile.TileContext(nc))
        wpool = ctx.enter_context(tc.tile_pool(name="wpool", bufs=1))
        xpool = ctx.enter_context(tc.tile_pool(name="xpool", bufs=4))
        ppool = ctx.enter_context(tc.tile_pool(name="ppool", bufs=2))
        spool = ctx.enter_context(tc.tile_pool(name="spool", bufs=1))

        wts = []
        for c in range(nch):
            wt = wpool.tile([P, chunk], f32, tag=f"w{c}")
            nc.sync.dma_start(wt[:], w[c])
            wts.append(wt)
        res = spool.tile([P, B_LOC * nch], f32)
        scratch = spool.tile([P, chunk], f32, tag="scratch")

        for b in range(B_LOC):
            for c in range(nch):
                xt = xpool.tile([P, chunk], f32)
                nc.sync.dma_start(xt[:], xs[b, c])
                prod = ppool.tile([P, chunk], f32)
                nc.vector.tensor_mul(prod[:], xt[:], wts[c][:])
                col = b * nch + c
                nc.scalar.activation(scratch[:], prod[:],
                                     mybir.ActivationFunctionType.Copy,
                                     accum_out=res[:, col:col + 1])

        nc.sync.dma_start(out[:], res[:])
    nc.compile()
    return nc


def _get_nc(keep: int):
    key = ("nc", keep)
    if key not in _CACHE:
        _CACHE[key] = _build_bass(keep)
    return _CACHE[key]


def kernel(x, state_matrix, input_matrix, output_matrix):
    global LAST_RESULTS
    from concourse.bass_utils import run_bass_kernel_spmd

    x = np.asarray(x, dtype=np.float32)
    assert x.shape == (B_FULL, T, D)
    w64 = _compute_w(state_matrix, input_matrix, output_matrix)
    w32 = np.ascontiguousarray(w64.astype(np.float32))
    keep = _pick_bucket(w32)
    forced = int(os.environ.get("BIMAMBA_FORCE_KEEP", "0"))
    if forced:
        assert forced in BUCKETS and forced >= keep
        keep = forced

    free = keep * D // P
    xt = x[:, T - keep:, :].reshape(B_FULL, P, free)

    if keep <= 256:
        wk = w32[T - keep:].reshape(P, free)
        # xb[c, b] = [P, free] view of batch b on core c
        xb = xt.reshape(N_CORES, B_LOC, P, free)

        def pack(c, b0, nb):
            return (xb[c, b0:b0 + nb].transpose(1, 0, 2)
                    .reshape(P, nb * free))

        in_maps = []
        for c in range(N_CORES):
            m = {}
            b0 = 0
            for i, nb in enumerate(_CHUNKS):
                xp = pack(c, b0, nb)
                if i == 0:
                    xp = np.concatenate([wk, xp], axis=1)
                m[f"xs{i}"] = np.ascontiguousarray(xp)
                b0 += nb
            in_maps.append(m)
    else:
        chunk = min(free, 2048)
        nch = free // chunk
        wk = np.ascontiguousarray(w32[T - keep:].reshape(nch, P, chunk))
        xk = np.ascontiguousarray(xt).reshape(N_CORES, B_LOC, nch, P, chunk)
        in_maps = [{"xs": xk[c], "w": wk} for c in range(N_CORES)]

    nc = _get_nc(keep)
    trace = bool(int(os.environ.get("BIMAMBA_TRACE", "0")))
    LAST_RESULTS = run_bass_kernel_spmd(
        nc, in_maps, list(range(N_CORES)), trace=trace)

    outs = []
    for c in range(N_CORES):
        res = LAST_RESULTS.results[c]["out"]  # [P, ncols]
        per_col = res.astype(np.float64).sum(axis=0)  # partition sums
        if keep <= 256:
            outs.append(per_col)  # already one column per batch
        else:
            nch = free // min(free, 2048)
            outs.append(per_col.reshape(B_LOC, nch).sum(axis=1))
    return np.concatenate(outs).reshape(B_FULL, 1).astype(np.float32)
